# revision 1
# baseline (speedup 1.0000x reference)
"""IoU / NMS-detection kernel for TRN2 (8 NeuronCores, data-parallel over batch).

Computes, for batch_boxes [32,8732,4] (cxcywh) and batch_gt [32,100,4]:
  ious [32,8732,100] f32, positive_mask = (iou>0.5)&valid, negative_mask = (iou<0.5)&valid

Device strategy (per core, 4 batches):
  - partitions = 128-anchor tiles (N padded 8732->8832 = 69*128), free dim = G=100
  - custom fused DVE ops:
      IOU_DX:    out = relu(min(px2, gx2) - max(px1, gx1))   (bitwise == reference)
      IOU_UNION: out = (area_p + area_g) - inter             (bitwise == reference)
  - inter = dxr*dyr (DVE tt), r = reciprocal_approx_accurate(union) (~2 ULP),
    iou = inter*r, m = ScalarE Sign(iou - 0.5) -> int8 in {-1,0,1}
  - host applies the valid mask to pos/neg (valid is known host-side), since
    invalid gt are made degenerate (coords -1e6, area 0) so iou == 0 exactly.
"""

import os
import numpy as np

import concourse.bacc as bacc
import concourse.mybir as mybir
import concourse.tile as tile
import concourse.dve_ops as dve_ops
from concourse.bass_utils import run_bass_kernel_spmd
from concourse.dve_spec import Spec, Src0, Src1, C0, C1, relu, minn, maxx, lower, _has_src1
from concourse.dve_uop import DveOpSpec

B, N, G = 32, 8732, 100
NCORES = 8
BPC = B // NCORES          # batches per core
NT = 69                    # anchor tiles per batch (padded)
NPAD = NT * 128            # 8832
K = 23                     # tiles per supertile
NST = NT // K              # supertiles per batch

_f32 = mybir.dt.float32
_s8 = mybir.dt.int8


def _register_op(name, spec):
    for op in dve_ops.OPS:
        if op.name == name:
            return op
    row = dve_ops._CUSTOM_DVE_ROW_BASE + len(dve_ops.OPS)
    assert row < 0x20
    dve_ops._SUB_OPCODE_FOR_NAME[name] = row
    sha3 = DveOpSpec(
        name=name, opcode=row, uops=lower(spec, ver="v3"), rd1_en=_has_src1(spec)
    ).sha("v3")
    op = dve_ops.DveOp(name, spec, False, {"v3": sha3})
    dve_ops.OPS.append(op)
    dve_ops.CUSTOM_DVE_SPECS[name] = spec
    return op


IOU_DX = _register_op(
    "IOU_DX_ANT",
    Spec(
        body=relu(minn(C0, Src0) - maxx(C1, Src1)),
        reference=lambda in0, in1, s0, s1, imm2: np.maximum(
            np.minimum(s0, in0.astype(np.float32)) - np.maximum(s1, in1), 0
        ).astype(np.float32),
    ),
)

IOU_UNION = _register_op(
    "IOU_UNION_ANT",
    Spec(
        body=(C0 + Src1) - Src0,
        reference=lambda in0, in1, s0, s1, imm2: (
            (s0 + in1.astype(np.float32)) - in0
        ).astype(np.float32),
    ),
)


_NC_CACHE = {}


def _build_nc():
    nc = bacc.Bacc("TRN2", target_bir_lowering=False, debug=False)
    pf = nc.dram_tensor("pf", [BPC, 128, NT * 5], _f32, kind="ExternalInput")
    gt = nc.dram_tensor("gt", [BPC, 128, 5 * G], _f32, kind="ExternalInput")
    # supertile-major layout: [b, st, p, K*G] -> per-partition contiguous runs
    iou_d = nc.dram_tensor("iou_out", [BPC, NST, 128, K * G], _f32, kind="ExternalOutput")
    m_d = nc.dram_tensor("m_out", [BPC, NST, 128, K * G], _s8, kind="ExternalOutput")

    with tile.TileContext(nc) as tc:
        with tc.tile_pool(name="const", bufs=1) as cpool, tc.tile_pool(
            name="io", bufs=2
        ) as iop, tc.tile_pool(name="st", bufs=2) as stp, tc.tile_pool(
            name="out", bufs=3
        ) as outp:
            neg_half = cpool.tile([128, 1], _f32, tag="neghalf")
            nc.vector.memset(neg_half[:], -0.5)
            for b in range(BPC):
                gt_t = iop.tile([128, 5 * G], _f32, tag="gt")
                pf_t = iop.tile([128, NT * 5], _f32, tag="pf")
                nc.sync.dma_start(out=gt_t[:], in_=gt[b])
                nc.sync.dma_start(out=pf_t[:], in_=pf[b])
                gx1 = gt_t[:, 0:G]
                gx2 = gt_t[:, G : 2 * G]
                gy1 = gt_t[:, 2 * G : 3 * G]
                gy2 = gt_t[:, 3 * G : 4 * G]
                ag = gt_t[:, 4 * G : 5 * G]
                for st in range(NST):
                    dxr = stp.tile([128, K * G], _f32, tag="dxr")
                    dyr = stp.tile([128, K * G], _f32, tag="dyr")
                    inter = stp.tile([128, K * G], _f32, tag="inter")
                    union = stp.tile([128, K * G], _f32, tag="union")
                    r0 = stp.tile([128, K * G], _f32, tag="r0")
                    r1 = stp.tile([128, K * G], _f32, tag="r1")
                    iou = outp.tile([128, K * G], _f32, tag="iou")
                    mm = outp.tile([128, K * G], _s8, tag="mm")
                    for i in range(K):
                        t = st * K + i
                        sl = slice(i * G, (i + 1) * G)
                        px1 = pf_t[:, t * 5 + 0 : t * 5 + 1]
                        px2 = pf_t[:, t * 5 + 1 : t * 5 + 2]
                        py1 = pf_t[:, t * 5 + 2 : t * 5 + 3]
                        py2 = pf_t[:, t * 5 + 3 : t * 5 + 4]
                        nc.vector._custom_dve(
                            IOU_DX, out=dxr[:, sl], in0=gx2, in1=gx1, s0=px2, s1=px1
                        )
                        nc.vector._custom_dve(
                            IOU_DX, out=dyr[:, sl], in0=gy2, in1=gy1, s0=py2, s1=py1
                        )
                    nc.vector.tensor_mul(inter[:], dxr[:], dyr[:])
                    for i in range(K):
                        t = st * K + i
                        sl = slice(i * G, (i + 1) * G)
                        apf = pf_t[:, t * 5 + 4 : t * 5 + 5]
                        nc.vector._custom_dve(
                            IOU_UNION, out=union[:, sl], in0=inter[:, sl], in1=ag, s0=apf
                        )
                    nc.vector.reciprocal_approx_accurate(
                        out=r1[:], in_=union[:], scratch=r0[:]
                    )
                    nc.vector.tensor_mul(iou[:], inter[:], r1[:])
                    nc.scalar.sign(out=mm[:], in_=iou[:], bias=neg_half[:])
                    nsplit = 4
                    step = (K * G) // nsplit  # 575
                    for s in range(nsplit):
                        fsl = slice(s * step, (s + 1) * step)
                        nc.sync.dma_start(
                            out=iou_d[b, st, :, fsl], in_=iou[:, fsl]
                        )
                    nc.sync.dma_start(out=m_d[b, st, :, :], in_=mm[:])
    nc.compile()
    return nc


def _get_nc():
    if "nc" not in _NC_CACHE:
        _NC_CACHE["nc"] = _build_nc()
    return _NC_CACHE["nc"]


def kernel(
    threshhold=None,
    batch_boxes=None,
    batch_classes=None,
    batch_gt=None,
    batch_num_objects=None,
    **_kw,
):
    boxes = np.asarray(batch_boxes, np.float32)
    gtb = np.asarray(batch_gt, np.float32)
    no = np.asarray(batch_num_objects).astype(np.int64)

    half = np.float32(0.5)
    cx, cy, w, h = boxes[..., 0], boxes[..., 1], boxes[..., 2], boxes[..., 3]
    px1 = cx - w * half
    py1 = cy - h * half
    px2 = cx + w * half
    py2 = cy + h * half
    area_p = (px2 - px1) * (py2 - py1)

    def pad(a, fill):
        out = np.full((B, NPAD), fill, np.float32)
        out[:, :N] = a
        return out

    pf = np.stack(
        [pad(px1, -1e4), pad(px2, -1e4), pad(py1, -1e4), pad(py2, -1e4), pad(area_p, 1.0)],
        axis=-1,
    )  # [B, NPAD, 5]
    pf = np.ascontiguousarray(
        pf.reshape(B, NT, 128, 5).transpose(0, 2, 1, 3).reshape(B, 128, NT * 5)
    )

    gcx, gcy, gw, gh = gtb[..., 0], gtb[..., 1], gtb[..., 2], gtb[..., 3]
    gx1 = gcx - gw * half
    gy1 = gcy - gh * half
    gx2 = gcx + gw * half
    gy2 = gcy + gh * half
    area_g = (gx2 - gx1) * (gy2 - gy1)
    validm = np.arange(G)[None, :] < no[:, None]  # [B, G]
    NEG = np.float32(-1e6)
    gx1 = np.where(validm, gx1, NEG).astype(np.float32)
    gx2 = np.where(validm, gx2, NEG).astype(np.float32)
    gy1 = np.where(validm, gy1, NEG).astype(np.float32)
    gy2 = np.where(validm, gy2, NEG).astype(np.float32)
    area_g = np.where(validm, area_g, np.float32(0.0)).astype(np.float32)
    gtpack = np.concatenate([gx1, gx2, gy1, gy2, area_g], axis=1)  # [B, 500]
    gtpack = np.ascontiguousarray(
        np.broadcast_to(gtpack[:, None, :], (B, 128, 5 * G))
    )

    nc = _get_nc()
    in_maps = [
        {
            "pf": np.ascontiguousarray(pf[c * BPC : (c + 1) * BPC]),
            "gt": np.ascontiguousarray(gtpack[c * BPC : (c + 1) * BPC]),
        }
        for c in range(NCORES)
    ]
    trace = os.environ.get("IOU_TRACE", "0") == "1"
    res = run_bass_kernel_spmd(nc, in_maps, list(range(NCORES)), trace=trace)
    _NC_CACHE["last_result"] = res
    results = res.results

    def unscramble(a):
        # [BPC, NST, 128, K*G] -> [BPC, NPAD, G]; anchor n = (st*K+i)*128 + p
        a = a.reshape(BPC, NST, 128, K, G).transpose(0, 1, 3, 2, 4)
        return a.reshape(BPC, NPAD, G)

    iou_full = np.concatenate([unscramble(r["iou_out"]) for r in results], axis=0)
    m_full = np.concatenate([unscramble(r["m_out"]) for r in results], axis=0)
    ious = np.ascontiguousarray(iou_full[:, :N, :])
    m = m_full[:, :N, :]
    vb = validm[:, None, :]
    pos = (m == 1) & vb
    neg = (m == -1) & vb
    return ious, pos, neg



# revision 2
# speedup vs baseline: 2.0026x; 2.0026x over previous
"""IoU / NMS-detection kernel v3 for TRN2 (8 NeuronCores, data-parallel).

Computes, for batch_boxes [32,8732,4] (cxcywh) and batch_gt [32,100,4]:
  ious [32,8732,100] f32, positive_mask = (iou>0.5)&valid, negative_mask = (iou<0.5)&valid

Structure (per core: 4 batch slots x 3 supertiles of 23 anchor-tiles):
  - DVE: per-tile overlap customs (dx, dy), inter = dx*dy,
    msub = 2*inter - union (scalar_tensor_tensor; Pool lacks that opcode).
  - Pool (gpsimd): u1 = ag - inter (broadcast tensor_tensor),
    union = u1 + ap (broadcast tt; one supertile per 12 runs on DVE),
    iou = inter * ru.
  - Act: Ln(union), ru = Exp(-ln) (reciprocal for the VALUE path; the mask
    path is exact via sign(msub)), Sign(msub) -> int8. One pinned act table
    holds Ln+Exp+Sign.
  - adaptive gt-count: batches sorted by num_objects into 4 per-core slots;
    slot s computes only G_s = max(num_objects in slot) gt columns (g-major
    [G, K] layout); the rest is zero-filled by DMA from a constant tile.
  - software pipeline: phase A(i+1) (customs/inter/u1) is emitted before
    phase B(i) (union/ln/exp/msub/iou/sign/DMA) to hide cross-engine latency.
"""

import os
import numpy as np

import concourse.bacc as bacc
import concourse.mybir as mybir
import concourse.tile as tile
import concourse.dve_ops as dve_ops
from concourse.bass_utils import run_bass_kernel_spmd
from concourse.dve_spec import Spec, Src0, Src1, C0, C1, relu, minn, maxx, lower, _has_src1
from concourse.dve_uop import DveOpSpec

B, N, G = 32, 8732, 100
NCORES = 8
BPC = B // NCORES          # batch slots per core
NT = 69                    # anchor tiles per batch (padded)
NPAD = NT * 128            # 8832
K = 23                     # tiles per supertile
NST = NT // K              # supertiles per batch
KG = K * G                 # full output row block per supertile

_f32 = mybir.dt.float32
_s8 = mybir.dt.int8
_ALU = mybir.AluOpType
_ACT = mybir.ActivationFunctionType


def _act_table_id():
    from concourse.hw_specs import get_activation_tables

    for idx, (nm, fns) in enumerate(get_activation_tables("gen3").items()):
        if (
            mybir.ActivationFunctionType.Ln in fns
            and mybir.ActivationFunctionType.Exp in fns
            and mybir.ActivationFunctionType.Sign in fns
        ):
            return idx
    raise RuntimeError("no act table with Ln+Exp+Sign")


ACT_TABLE_ID = _act_table_id()


def _register_op(name, spec):
    for op in dve_ops.OPS:
        if op.name == name:
            return op
    row = dve_ops._CUSTOM_DVE_ROW_BASE + len(dve_ops.OPS)
    assert row < 0x20
    dve_ops._SUB_OPCODE_FOR_NAME[name] = row
    sha3 = DveOpSpec(
        name=name, opcode=row, uops=lower(spec, ver="v3"), rd1_en=_has_src1(spec)
    ).sha("v3")
    op = dve_ops.DveOp(name, spec, False, {"v3": sha3})
    dve_ops.OPS.append(op)
    dve_ops.CUSTOM_DVE_SPECS[name] = spec
    return op


IOU_DX = _register_op(
    "IOU_DX_ANT",
    Spec(
        body=relu(minn(C0, Src0) - maxx(C1, Src1)),
        reference=lambda in0, in1, s0, s1, imm2: np.maximum(
            np.minimum(s0, in0.astype(np.float32)) - np.maximum(s1, in1), 0
        ).astype(np.float32),
    ),
)


_NC_CACHE = {}


def _build_nc(gs):
    """gs: tuple of 4 per-slot gt counts (each <= 100)."""
    nc = bacc.Bacc("TRN2", target_bir_lowering=False, debug=False)
    pf = nc.dram_tensor("pf", [BPC, 128, NT * 5], _f32, kind="ExternalInput")
    gt = nc.dram_tensor("gt", [BPC, 128, 5 * G], _f32, kind="ExternalInput")
    # g-major supertile layout: [slot, st, p, g*K + k]; anchor n = (st*K+k)*128 + p
    iou_d = nc.dram_tensor("iou_out", [BPC, NST, 128, KG], _f32, kind="ExternalOutput")
    m_d = nc.dram_tensor("m_out", [BPC, NST, 128, KG], _s8, kind="ExternalOutput")

    with tile.TileContext(nc) as tc:
        with tc.tile_pool(name="const", bufs=1) as cpool, tc.tile_pool(
            name="io", bufs=2
        ) as iop, tc.tile_pool(name="st", bufs=3) as stp, tc.tile_pool(
            name="out", bufs=3
        ) as outp:
            # pin the act table that holds Ln+Exp+Sign so the auto-inserter
            # doesn't ping-pong between per-func tables each supertile
            _actload = mybir.InstLoadActFuncSet(
                name=nc.get_next_instruction_name(), ins=[], outs=[],
                act_func_set_id=ACT_TABLE_ID,
            )
            _actload.engine = mybir.EngineType.Activation
            nc.scalar.add_instruction(_actload)
            zspan = max(1, KG - min(gs) * K)
            zf = cpool.tile([128, zspan], _f32, tag="zf")
            zi = cpool.tile([128, zspan], _s8, tag="zi")
            nc.gpsimd.memset(zf[:], 0.0)
            nc.gpsimd.memset(zi[:], 0)

            io_tiles = {}

            def load_io(s):
                g = gs[s]
                gt_t = iop.tile([128, 5 * G], _f32, tag="gt")
                pf_t = iop.tile([128, NT * 5], _f32, tag="pf")
                nc.sync.dma_start(out=gt_t[:, : 5 * g], in_=gt[s, :, : 5 * g])
                nc.sync.dma_start(out=pf_t[:], in_=pf[s])
                io_tiles[s] = (gt_t, pf_t)

            def phase_a(s, st):
                """overlap customs + inter (DVE) + u1 (Pool)."""
                g = gs[s]
                fs = g * K
                gt_t, pf_t = io_tiles[s]
                gx1 = gt_t[:, 0:g]
                gx2 = gt_t[:, g : 2 * g]
                gy1 = gt_t[:, 2 * g : 3 * g]
                gy2 = gt_t[:, 3 * g : 4 * g]
                ag_b = gt_t[:, 4 * g : 5 * g].unsqueeze(2).broadcast_to([128, g, K])
                dxr = stp.tile([128, fs], _f32, tag="dxr")
                dyr = stp.tile([128, fs], _f32, tag="dyr")
                inter = stp.tile([128, fs], _f32, tag="inter")
                dxr3 = dxr[:].rearrange("p (g k) -> p g k", k=K)
                dyr3 = dyr[:].rearrange("p (g k) -> p g k", k=K)
                for k in range(K):
                    t = st * K + k
                    px1 = pf_t[:, t * 5 + 0 : t * 5 + 1]
                    px2 = pf_t[:, t * 5 + 1 : t * 5 + 2]
                    py1 = pf_t[:, t * 5 + 2 : t * 5 + 3]
                    py2 = pf_t[:, t * 5 + 3 : t * 5 + 4]
                    nc.vector._custom_dve(
                        IOU_DX, out=dxr3[:, :, k : k + 1].squeeze(2),
                        in0=gx2, in1=gx1, s0=px2, s1=px1,
                    )
                    nc.vector._custom_dve(
                        IOU_DX, out=dyr3[:, :, k : k + 1].squeeze(2),
                        in0=gy2, in1=gy1, s0=py2, s1=py1,
                    )
                nc.vector.tensor_mul(inter[:], dxr[:], dyr[:])
                return dxr, dyr, inter

            def phase_u1(s, tiles):
                g = gs[s]
                gt_t, _ = io_tiles[s]
                ag_b = gt_t[:, 4 * g : 5 * g].unsqueeze(2).broadcast_to([128, g, K])
                dxr, dyr, inter = tiles
                inter3 = inter[:].rearrange("p (g k) -> p g k", k=K)
                u13 = dxr[:].rearrange("p (g k) -> p g k", k=K)  # reuse dxr
                # u1 = ag - inter  (Pool tt; stt has no Pool encoding)
                nc.gpsimd.tensor_tensor(u13, ag_b, inter3, _ALU.subtract)

            def phase_b1(s, st, sti, tiles):
                """union + msub + ln + exp — Pool slot `union` then Act."""
                g = gs[s]
                fs = g * K
                _, pf_t = io_tiles[s]
                dxr, dyr, inter = tiles
                u13 = dxr[:].rearrange("p (g k) -> p g k", k=K)
                union = stp.tile([128, fs], _f32, tag="union")
                apf_b = (
                    pf_t[:, st * K * 5 + 4 : (st + 1) * K * 5 : 5]
                    .unsqueeze(1)
                    .broadcast_to([128, g, K])
                )
                union3 = union[:].rearrange("p (g k) -> p g k", k=K)
                ueng = nc.vector if sti % 12 == 5 else nc.gpsimd
                ueng.tensor_tensor(union3, u13, apf_b, _ALU.add)
                # mask path: msub = 2*inter - union, exact sign (DVE stt)
                msub = stp.tile([128, fs], _f32, tag="msub")
                nc.vector.scalar_tensor_tensor(
                    msub[:], inter[:], 2.0, union[:], _ALU.mult, _ALU.subtract
                )
                # value path: iou = inter * exp(-ln(union))
                lnu = dyr  # dyr dead after inter
                nc.scalar.activation(lnu[:], union[:], _ACT.Ln)
                ru = dxr  # dxr (u1) dead after union
                nc.scalar.activation(ru[:], lnu[:], _ACT.Exp, scale=-1.0)
                return msub, ru

            def phase_b2(s, st, tiles, btiles):
                g = gs[s]
                fs = g * K
                _, _, inter = tiles
                msub, ru = btiles
                iou = outp.tile([128, fs], _f32, tag="iou")
                mm = outp.tile([128, fs], _s8, tag="mm")
                nc.gpsimd.tensor_mul(iou[:], inter[:], ru[:])
                nc.scalar.sign(out=mm[:], in_=msub[:])
                nc.sync.dma_start(out=iou_d[s, st, :, 0:fs], in_=iou[:])
                nc.sync.dma_start(out=m_d[s, st, :, 0:fs], in_=mm[:])
                if fs < KG:
                    nc.sync.dma_start(
                        out=iou_d[s, st, :, fs:KG], in_=zf[:, : KG - fs]
                    )
                    nc.sync.dma_start(
                        out=m_d[s, st, :, fs:KG], in_=zi[:, : KG - fs]
                    )

            # software pipeline with one-supertile skew. Pool's stream per
            # iteration is [union(i), u1(i+1), iou(i)] so the Act ln/exp
            # round-trip before iou(i) is hidden behind u1(i+1).
            slot_order = list(range(BPC))
            order_st = [(s, st) for s in slot_order for st in range(NST)]
            load_io(slot_order[0])
            pending = None
            for i, (s, st) in enumerate(order_st):
                if st == NST - 1 and i + 1 < len(order_st):
                    load_io(order_st[i + 1][0])
                tiles = phase_a(s, st)
                if pending is not None:
                    ps, pst, ptiles, pbtiles = pending
                    pbtiles = phase_b1(ps, pst, ps * NST + pst, ptiles)
                    phase_u1(s, tiles)
                    phase_b2(ps, pst, ptiles, pbtiles)
                else:
                    phase_u1(s, tiles)
                pending = (s, st, tiles, None)
            ps, pst, ptiles, _ = pending
            pbtiles = phase_b1(ps, pst, ps * NST + pst, ptiles)
            phase_b2(ps, pst, ptiles, pbtiles)
    nc.compile()
    return nc


def _get_nc(gs):
    key = tuple(gs)
    if key not in _NC_CACHE:
        _NC_CACHE[key] = _build_nc(key)
    return _NC_CACHE[key]


def kernel(
    threshhold=None,
    batch_boxes=None,
    batch_classes=None,
    batch_gt=None,
    batch_num_objects=None,
    **_kw,
):
    boxes = np.asarray(batch_boxes, np.float32)
    gtb = np.asarray(batch_gt, np.float32)
    no = np.asarray(batch_num_objects).astype(np.int64)

    half = np.float32(0.5)
    cx, cy, w, h = boxes[..., 0], boxes[..., 1], boxes[..., 2], boxes[..., 3]
    px1 = cx - w * half
    py1 = cy - h * half
    px2 = cx + w * half
    py2 = cy + h * half
    area_p = (px2 - px1) * (py2 - py1)

    def pad(a, fill):
        out = np.full((B, NPAD), fill, np.float32)
        out[:, :N] = a
        return out

    pf = np.stack(
        [pad(px1, -1e4), pad(px2, -1e4), pad(py1, -1e4), pad(py2, -1e4), pad(area_p, 1.0)],
        axis=-1,
    )  # [B, NPAD, 5]
    pf = np.ascontiguousarray(
        pf.reshape(B, NT, 128, 5).transpose(0, 2, 1, 3).reshape(B, 128, NT * 5)
    )

    gcx, gcy, gw, gh = gtb[..., 0], gtb[..., 1], gtb[..., 2], gtb[..., 3]
    gx1 = gcx - gw * half
    gy1 = gcy - gh * half
    gx2 = gcx + gw * half
    gy2 = gcy + gh * half
    area_g = (gx2 - gx1) * (gy2 - gy1)
    validm = np.arange(G)[None, :] < no[:, None]  # [B, G]
    NEG = np.float32(-1e6)
    gx1 = np.where(validm, gx1, NEG).astype(np.float32)
    gx2 = np.where(validm, gx2, NEG).astype(np.float32)
    gy1 = np.where(validm, gy1, NEG).astype(np.float32)
    gy2 = np.where(validm, gy2, NEG).astype(np.float32)
    area_g = np.where(validm, area_g, np.float32(0.0)).astype(np.float32)

    # sort batches by num_objects desc; slot s takes ranks [s*8, s*8+8)
    order = np.argsort(-no, kind="stable")
    gs = []
    for s in range(BPC):
        mx = int(no[order[s * NCORES : (s + 1) * NCORES]].max())
        mx = min(G, max(8, -(-mx // 4) * 4))
        gs.append(mx)
    gs = tuple(gs)

    # gt pack per batch: [gx1 | gx2 | gy1 | gy2 | ag] each g_s wide
    gtpack = np.zeros((B, 5 * G), np.float32)
    slot_of = np.empty(B, np.int64)
    for rank, b in enumerate(order):
        slot_of[b] = rank // NCORES
    for b in range(B):
        g = gs[slot_of[b]]
        gtpack[b, 0 * g : 1 * g] = gx1[b, :g]
        gtpack[b, 1 * g : 2 * g] = gx2[b, :g]
        gtpack[b, 2 * g : 3 * g] = gy1[b, :g]
        gtpack[b, 3 * g : 4 * g] = gy2[b, :g]
        gtpack[b, 4 * g : 5 * g] = area_g[b, :g]
    gtrep = np.broadcast_to(gtpack[:, None, :], (B, 128, 5 * G))

    nc = _get_nc(gs)
    in_maps = []
    for c in range(NCORES):
        bidx = [int(order[s * NCORES + c]) for s in range(BPC)]
        in_maps.append(
            {
                "pf": np.ascontiguousarray(pf[bidx]),
                "gt": np.ascontiguousarray(gtrep[bidx]),
            }
        )
    trace = os.environ.get("IOU_TRACE", "0") == "1"
    res = run_bass_kernel_spmd(nc, in_maps, list(range(NCORES)), trace=trace)
    _NC_CACHE["last_result"] = res
    results = res.results

    def unscramble(a):
        # [BPC, NST, 128, G*K] g-major -> [BPC, NPAD, G]; n = (st*K+k)*128 + p
        a = a.reshape(BPC, NST, 128, G, K).transpose(0, 1, 4, 2, 3)
        return a.reshape(BPC, NPAD, G)

    iou_full = np.empty((B, N, G), np.float32)
    m_full = np.empty((B, N, G), np.int8)
    for c in range(NCORES):
        r = results[c]
        iu = unscramble(r["iou_out"])
        mu = unscramble(r["m_out"])
        for s in range(BPC):
            b = int(order[s * NCORES + c])
            iou_full[b] = iu[s, :N]
            m_full[b] = mu[s, :N]
    vb = validm[:, None, :]
    pos = (m_full > 0) & vb
    neg = (m_full < 0) & vb
    return iou_full, pos, neg


# revision 3
# speedup vs baseline: 2.0118x; 1.0046x over previous
"""IoU / NMS-detection kernel v3 for TRN2 (8 NeuronCores, data-parallel).

Computes, for batch_boxes [32,8732,4] (cxcywh) and batch_gt [32,100,4]:
  ious [32,8732,100] f32, positive_mask = (iou>0.5)&valid, negative_mask = (iou<0.5)&valid

Structure (per core: 4 batch slots x 3 supertiles of 23 anchor-tiles):
  - DVE: per-tile overlap customs (dx, dy), inter = dx*dy,
    msub = 2*inter - union (scalar_tensor_tensor; Pool lacks that opcode).
  - Pool (gpsimd): u1 = ag - inter (broadcast tensor_tensor),
    union = u1 + ap (broadcast tt; one supertile per 12 runs on DVE),
    iou = inter * ru.
  - Act: Ln(union), ru = Exp(-ln) (reciprocal for the VALUE path; the mask
    path is exact via sign(msub)), Sign(msub) -> int8. One pinned act table
    holds Ln+Exp+Sign.
  - adaptive gt-count: batches sorted by num_objects into 4 per-core slots;
    slot s computes only G_s = max(num_objects in slot) gt columns (g-major
    [G, K] layout); the rest is zero-filled by DMA from a constant tile.
  - software pipeline: phase A(i+1) (customs/inter/u1) is emitted before
    phase B(i) (union/ln/exp/msub/iou/sign/DMA) to hide cross-engine latency.
"""

import os
import numpy as np

import concourse.bacc as bacc
import concourse.mybir as mybir
import concourse.tile as tile
import concourse.dve_ops as dve_ops
from concourse.bass_utils import run_bass_kernel_spmd
from concourse.dve_spec import Spec, Src0, Src1, C0, C1, relu, minn, maxx, lower, _has_src1
from concourse.dve_uop import DveOpSpec

B, N, G = 32, 8732, 100
NCORES = 8
BPC = B // NCORES          # batch slots per core
NT = 69                    # anchor tiles per batch (padded)
NPAD = NT * 128            # 8832
K = 23                     # tiles per supertile
NST = NT // K              # supertiles per batch
KG = K * G                 # full output row block per supertile

_f32 = mybir.dt.float32
_s8 = mybir.dt.int8
_ALU = mybir.AluOpType
_ACT = mybir.ActivationFunctionType


def _act_table_id():
    from concourse.hw_specs import get_activation_tables

    for idx, (nm, fns) in enumerate(get_activation_tables("gen3").items()):
        if (
            mybir.ActivationFunctionType.Ln in fns
            and mybir.ActivationFunctionType.Exp in fns
            and mybir.ActivationFunctionType.Sign in fns
        ):
            return idx
    raise RuntimeError("no act table with Ln+Exp+Sign")


ACT_TABLE_ID = _act_table_id()


def _register_op(name, spec):
    for op in dve_ops.OPS:
        if op.name == name:
            return op
    row = dve_ops._CUSTOM_DVE_ROW_BASE + len(dve_ops.OPS)
    assert row < 0x20
    dve_ops._SUB_OPCODE_FOR_NAME[name] = row
    sha3 = DveOpSpec(
        name=name, opcode=row, uops=lower(spec, ver="v3"), rd1_en=_has_src1(spec)
    ).sha("v3")
    op = dve_ops.DveOp(name, spec, False, {"v3": sha3})
    dve_ops.OPS.append(op)
    dve_ops.CUSTOM_DVE_SPECS[name] = spec
    return op


IOU_DX = _register_op(
    "IOU_DX_ANT",
    Spec(
        body=relu(minn(C0, Src0) - maxx(C1, Src1)),
        reference=lambda in0, in1, s0, s1, imm2: np.maximum(
            np.minimum(s0, in0.astype(np.float32)) - np.maximum(s1, in1), 0
        ).astype(np.float32),
    ),
)


_NC_CACHE = {}


def _build_nc(gs):
    """gs: tuple of 4 per-slot gt counts (each <= 100)."""
    nc = bacc.Bacc("TRN2", target_bir_lowering=False, debug=False)
    pf = nc.dram_tensor("pf", [BPC, 128, NT * 5], _f32, kind="ExternalInput")
    gt = nc.dram_tensor("gt", [BPC, 128, 5 * G], _f32, kind="ExternalInput")
    # g-major supertile layout: [slot, st, p, g*K + k]; anchor n = (st*K+k)*128 + p
    iou_d = nc.dram_tensor("iou_out", [BPC, NST, 128, KG], _f32, kind="ExternalOutput")
    m_d = nc.dram_tensor("m_out", [BPC, NST, 128, KG], _s8, kind="ExternalOutput")

    with tile.TileContext(nc) as tc:
        with tc.tile_pool(name="const", bufs=1) as cpool, tc.tile_pool(
            name="io", bufs=2
        ) as iop, tc.tile_pool(name="st", bufs=3) as stp, tc.tile_pool(
            name="out", bufs=3
        ) as outp:
            # pin the act table that holds Ln+Exp+Sign so the auto-inserter
            # doesn't ping-pong between per-func tables each supertile
            _actload = mybir.InstLoadActFuncSet(
                name=nc.get_next_instruction_name(), ins=[], outs=[],
                act_func_set_id=ACT_TABLE_ID,
            )
            _actload.engine = mybir.EngineType.Activation
            nc.scalar.add_instruction(_actload)
            zspan = max(1, KG - min(gs) * K)
            zf = cpool.tile([128, zspan], _f32, tag="zf")
            zi = cpool.tile([128, zspan], _s8, tag="zi")
            nc.gpsimd.memset(zf[:], 0.0)
            nc.gpsimd.memset(zi[:], 0)

            io_tiles = {}

            def load_io(s):
                g = gs[s]
                gt_t = iop.tile([128, 5 * G], _f32, tag="gt")
                pf_t = iop.tile([128, NT * 5], _f32, tag="pf")
                nc.sync.dma_start(out=gt_t[:, : 5 * g], in_=gt[s, :, : 5 * g])
                nc.sync.dma_start(out=pf_t[:], in_=pf[s])
                io_tiles[s] = (gt_t, pf_t)

            def phase_a(s, st):
                """overlap customs + inter (DVE) + u1 (Pool)."""
                g = gs[s]
                fs = g * K
                gt_t, pf_t = io_tiles[s]
                gx1 = gt_t[:, 0:g]
                gx2 = gt_t[:, g : 2 * g]
                gy1 = gt_t[:, 2 * g : 3 * g]
                gy2 = gt_t[:, 3 * g : 4 * g]
                ag_b = gt_t[:, 4 * g : 5 * g].unsqueeze(2).broadcast_to([128, g, K])
                dxr = stp.tile([128, fs], _f32, tag="dxr")
                dyr = stp.tile([128, fs], _f32, tag="dyr")
                inter = stp.tile([128, fs], _f32, tag="inter")
                dxr3 = dxr[:].rearrange("p (g k) -> p g k", k=K)
                dyr3 = dyr[:].rearrange("p (g k) -> p g k", k=K)
                for k in range(K):
                    t = st * K + k
                    px1 = pf_t[:, t * 5 + 0 : t * 5 + 1]
                    px2 = pf_t[:, t * 5 + 1 : t * 5 + 2]
                    py1 = pf_t[:, t * 5 + 2 : t * 5 + 3]
                    py2 = pf_t[:, t * 5 + 3 : t * 5 + 4]
                    nc.vector._custom_dve(
                        IOU_DX, out=dxr3[:, :, k : k + 1].squeeze(2),
                        in0=gx2, in1=gx1, s0=px2, s1=px1,
                    )
                    nc.vector._custom_dve(
                        IOU_DX, out=dyr3[:, :, k : k + 1].squeeze(2),
                        in0=gy2, in1=gy1, s0=py2, s1=py1,
                    )
                nc.vector.tensor_mul(inter[:], dxr[:], dyr[:])
                return dxr, dyr, inter

            def phase_apg(s, st):
                """apg = ap + ag from broadcast APs — depends only on the
                slot's input tiles, so the scheduler can run it whenever the
                Pool engine would otherwise stall."""
                g = gs[s]
                fs = g * K
                gt_t, pf_t = io_tiles[s]
                ag_b = gt_t[:, 4 * g : 5 * g].unsqueeze(2).broadcast_to([128, g, K])
                apf_b = (
                    pf_t[:, st * K * 5 + 4 : (st + 1) * K * 5 : 5]
                    .unsqueeze(1)
                    .broadcast_to([128, g, K])
                )
                apg = stp.tile([128, fs], _f32, tag="apg")
                apg3 = apg[:].rearrange("p (g k) -> p g k", k=K)
                nc.gpsimd.tensor_tensor(apg3, apf_b, ag_b, _ALU.add)
                return apg

            def phase_b1(s, st, sti, tiles, apg):
                """union + msub + ln + exp."""
                g = gs[s]
                fs = g * K
                dxr, dyr, inter = tiles
                union = stp.tile([128, fs], _f32, tag="union")
                # union = (ap + ag) - inter — same rounding order as the
                # reference
                ueng = nc.vector if sti % 12 == 5 else nc.gpsimd
                ueng.tensor_tensor(union[:], apg[:], inter[:], _ALU.subtract)
                # mask path: msub = 2*inter - union, exact sign (DVE stt)
                msub = dyr  # dyr dead after inter
                nc.vector.scalar_tensor_tensor(
                    msub[:], inter[:], 2.0, union[:], _ALU.mult, _ALU.subtract
                )
                # value path: iou = inter * exp(-ln(union)); lnu lands in the
                # iou output tile, ru overwrites union (Ln was its last reader)
                iou = outp.tile([128, fs], _f32, tag="iou")
                lnu = iou
                nc.scalar.activation(lnu[:], union[:], _ACT.Ln)
                ru = union
                nc.scalar.activation(ru[:], lnu[:], _ACT.Exp, scale=-1.0)
                return msub, ru, iou

            def phase_b2(s, st, tiles, btiles):
                g = gs[s]
                fs = g * K
                _, _, inter = tiles
                msub, ru, iou = btiles
                mm = outp.tile([128, fs], _s8, tag="mm")
                nc.gpsimd.tensor_mul(iou[:], inter[:], ru[:])
                nc.scalar.sign(out=mm[:], in_=msub[:])
                nc.sync.dma_start(out=iou_d[s, st, :, 0:fs], in_=iou[:])
                nc.sync.dma_start(out=m_d[s, st, :, 0:fs], in_=mm[:])
                if fs < KG:
                    nc.sync.dma_start(
                        out=iou_d[s, st, :, fs:KG], in_=zf[:, : KG - fs]
                    )
                    nc.sync.dma_start(
                        out=m_d[s, st, :, fs:KG], in_=zi[:, : KG - fs]
                    )

            # software pipeline with one-supertile skew; apg(i) is emitted an
            # iteration early as dependency-free Pool filler.
            slot_order = list(range(BPC))
            order_st = [(s, st) for s in slot_order for st in range(NST)]
            load_io(slot_order[0])
            apgs = {0: phase_apg(*order_st[0])}
            pending = None
            for i, (s, st) in enumerate(order_st):
                if st == NST - 1 and i + 1 < len(order_st):
                    load_io(order_st[i + 1][0])
                tiles = phase_a(s, st)
                if i + 1 < len(order_st):
                    apgs[i + 1] = phase_apg(*order_st[i + 1])
                if pending is not None:
                    pi, ps, pst, ptiles = pending
                    pbtiles = phase_b1(ps, pst, ps * NST + pst, ptiles, apgs.pop(pi))
                    phase_b2(ps, pst, ptiles, pbtiles)
                pending = (i, s, st, tiles)
            pi, ps, pst, ptiles = pending
            pbtiles = phase_b1(ps, pst, ps * NST + pst, ptiles, apgs.pop(pi))
            phase_b2(ps, pst, ptiles, pbtiles)
    nc.compile()
    return nc


def _get_nc(gs):
    key = tuple(gs)
    if key not in _NC_CACHE:
        _NC_CACHE[key] = _build_nc(key)
    return _NC_CACHE[key]


def kernel(
    threshhold=None,
    batch_boxes=None,
    batch_classes=None,
    batch_gt=None,
    batch_num_objects=None,
    **_kw,
):
    boxes = np.asarray(batch_boxes, np.float32)
    gtb = np.asarray(batch_gt, np.float32)
    no = np.asarray(batch_num_objects).astype(np.int64)

    half = np.float32(0.5)
    cx, cy, w, h = boxes[..., 0], boxes[..., 1], boxes[..., 2], boxes[..., 3]
    px1 = cx - w * half
    py1 = cy - h * half
    px2 = cx + w * half
    py2 = cy + h * half
    area_p = (px2 - px1) * (py2 - py1)

    def pad(a, fill):
        out = np.full((B, NPAD), fill, np.float32)
        out[:, :N] = a
        return out

    pf = np.stack(
        [pad(px1, -1e4), pad(px2, -1e4), pad(py1, -1e4), pad(py2, -1e4), pad(area_p, 1.0)],
        axis=-1,
    )  # [B, NPAD, 5]
    pf = np.ascontiguousarray(
        pf.reshape(B, NT, 128, 5).transpose(0, 2, 1, 3).reshape(B, 128, NT * 5)
    )

    gcx, gcy, gw, gh = gtb[..., 0], gtb[..., 1], gtb[..., 2], gtb[..., 3]
    gx1 = gcx - gw * half
    gy1 = gcy - gh * half
    gx2 = gcx + gw * half
    gy2 = gcy + gh * half
    area_g = (gx2 - gx1) * (gy2 - gy1)
    validm = np.arange(G)[None, :] < no[:, None]  # [B, G]
    NEG = np.float32(-1e6)
    gx1 = np.where(validm, gx1, NEG).astype(np.float32)
    gx2 = np.where(validm, gx2, NEG).astype(np.float32)
    gy1 = np.where(validm, gy1, NEG).astype(np.float32)
    gy2 = np.where(validm, gy2, NEG).astype(np.float32)
    area_g = np.where(validm, area_g, np.float32(0.0)).astype(np.float32)

    # sort batches by num_objects desc; slot s takes ranks [s*8, s*8+8)
    order = np.argsort(-no, kind="stable")
    gs = []
    for s in range(BPC):
        mx = int(no[order[s * NCORES : (s + 1) * NCORES]].max())
        mx = min(G, max(8, -(-mx // 4) * 4))
        gs.append(mx)
    gs = tuple(gs)

    # gt pack per batch: [gx1 | gx2 | gy1 | gy2 | ag] each g_s wide
    gtpack = np.zeros((B, 5 * G), np.float32)
    slot_of = np.empty(B, np.int64)
    for rank, b in enumerate(order):
        slot_of[b] = rank // NCORES
    for b in range(B):
        g = gs[slot_of[b]]
        gtpack[b, 0 * g : 1 * g] = gx1[b, :g]
        gtpack[b, 1 * g : 2 * g] = gx2[b, :g]
        gtpack[b, 2 * g : 3 * g] = gy1[b, :g]
        gtpack[b, 3 * g : 4 * g] = gy2[b, :g]
        gtpack[b, 4 * g : 5 * g] = area_g[b, :g]
    gtrep = np.broadcast_to(gtpack[:, None, :], (B, 128, 5 * G))

    nc = _get_nc(gs)
    in_maps = []
    for c in range(NCORES):
        bidx = [int(order[s * NCORES + c]) for s in range(BPC)]
        in_maps.append(
            {
                "pf": np.ascontiguousarray(pf[bidx]),
                "gt": np.ascontiguousarray(gtrep[bidx]),
            }
        )
    trace = os.environ.get("IOU_TRACE", "0") == "1"
    res = run_bass_kernel_spmd(nc, in_maps, list(range(NCORES)), trace=trace)
    _NC_CACHE["last_result"] = res
    results = res.results

    def unscramble(a):
        # [BPC, NST, 128, G*K] g-major -> [BPC, NPAD, G]; n = (st*K+k)*128 + p
        a = a.reshape(BPC, NST, 128, G, K).transpose(0, 1, 4, 2, 3)
        return a.reshape(BPC, NPAD, G)

    iou_full = np.empty((B, N, G), np.float32)
    m_full = np.empty((B, N, G), np.int8)
    for c in range(NCORES):
        r = results[c]
        iu = unscramble(r["iou_out"])
        mu = unscramble(r["m_out"])
        for s in range(BPC):
            b = int(order[s * NCORES + c])
            iou_full[b] = iu[s, :N]
            m_full[b] = mu[s, :N]
    vb = validm[:, None, :]
    pos = (m_full > 0) & vb
    neg = (m_full < 0) & vb
    return iou_full, pos, neg


# revision 4
# speedup vs baseline: 2.2836x; 1.1351x over previous
"""IoU / NMS-detection kernel v3 for TRN2 (8 NeuronCores, data-parallel).

Computes, for batch_boxes [32,8732,4] (cxcywh) and batch_gt [32,100,4]:
  ious [32,8732,100] f32, positive_mask = (iou>0.5)&valid, negative_mask = (iou<0.5)&valid

Structure (per core: 4 batch slots x 3 supertiles of 23 anchor-tiles):
  - DVE: per-tile overlap customs (dx, dy), inter = dx*dy,
    msub = 2*inter - union (scalar_tensor_tensor; Pool lacks that opcode).
  - Pool (gpsimd): u1 = ag - inter (broadcast tensor_tensor),
    union = u1 + ap (broadcast tt; one supertile per 12 runs on DVE),
    iou = inter * ru.
  - Act: Ln(union), ru = Exp(-ln) (reciprocal for the VALUE path; the mask
    path is exact via sign(msub)), Sign(msub) -> int8. One pinned act table
    holds Ln+Exp+Sign.
  - adaptive gt-count: batches sorted by num_objects into 4 per-core slots;
    slot s computes only G_s = max(num_objects in slot) gt columns (g-major
    [G, K] layout); the rest is zero-filled by DMA from a constant tile.
  - software pipeline: phase A(i+1) (customs/inter/u1) is emitted before
    phase B(i) (union/ln/exp/msub/iou/sign/DMA) to hide cross-engine latency.
"""

import os
import numpy as np

import concourse.bacc as bacc
import concourse.mybir as mybir
import concourse.tile as tile
import concourse.dve_ops as dve_ops
from concourse.bass_utils import run_bass_kernel_spmd
from concourse.dve_spec import Spec, Src0, Src1, C0, C1, relu, minn, maxx, lower, _has_src1
from concourse.dve_uop import DveOpSpec

B, N, G = 32, 8732, 100
NCORES = 8
BPC = B // NCORES          # batch slots per core
NT = 69                    # anchor tiles per batch (padded)
NPAD = NT * 128            # 8832
K = 23                     # tiles per supertile
NST = NT // K              # supertiles per batch
KG = K * G                 # full output row block per supertile

_f32 = mybir.dt.float32
_s8 = mybir.dt.int8
_ALU = mybir.AluOpType
_ACT = mybir.ActivationFunctionType


def _act_table_id():
    from concourse.hw_specs import get_activation_tables

    for idx, (nm, fns) in enumerate(get_activation_tables("gen3").items()):
        if (
            mybir.ActivationFunctionType.Ln in fns
            and mybir.ActivationFunctionType.Exp in fns
            and mybir.ActivationFunctionType.Sign in fns
        ):
            return idx
    raise RuntimeError("no act table with Ln+Exp+Sign")


ACT_TABLE_ID = _act_table_id()


def _register_op(name, spec):
    for op in dve_ops.OPS:
        if op.name == name:
            return op
    row = dve_ops._CUSTOM_DVE_ROW_BASE + len(dve_ops.OPS)
    assert row < 0x20
    dve_ops._SUB_OPCODE_FOR_NAME[name] = row
    sha3 = DveOpSpec(
        name=name, opcode=row, uops=lower(spec, ver="v3"), rd1_en=_has_src1(spec)
    ).sha("v3")
    op = dve_ops.DveOp(name, spec, False, {"v3": sha3})
    dve_ops.OPS.append(op)
    dve_ops.CUSTOM_DVE_SPECS[name] = spec
    return op


IOU_DX = _register_op(
    "IOU_DX_ANT",
    Spec(
        body=relu(minn(C0, Src0) - maxx(C1, Src1)),
        reference=lambda in0, in1, s0, s1, imm2: np.maximum(
            np.minimum(s0, in0.astype(np.float32)) - np.maximum(s1, in1), 0
        ).astype(np.float32),
    ),
)

from concourse.dve_spec import Bin, AluOp

_m2i = Bin(AluOp.MULTIPLY, C0, Src0)
IOU_MSIGN = _register_op(
    "IOU_MSIGN_ANT",
    Spec(
        body=Bin(AluOp.SUBTRACT, Bin(AluOp.IS_LT, Src1, _m2i), Bin(AluOp.IS_LT, _m2i, Src1)),
        reference=lambda in0, in1, s0, s1, imm2: (
            (in1 < s0 * in0).astype(np.float32) - (s0 * in0 < in1)
        ).astype(np.float32),
    ),
)


_NC_CACHE = {}


def _build_nc(gs):
    """gs: tuple of 4 per-slot gt counts (each <= 100)."""
    nc = bacc.Bacc("TRN2", target_bir_lowering=False, debug=False)
    pf = nc.dram_tensor("pf", [BPC, 128, NT * 5], _f32, kind="ExternalInput")
    gt = nc.dram_tensor("gt", [BPC, 128, 5 * G], _f32, kind="ExternalInput")
    # g-major supertile layout: [slot, st, p, g*K + k]; anchor n = (st*K+k)*128 + p
    iou_d = nc.dram_tensor("iou_out", [BPC, NST, 128, KG], _f32, kind="ExternalOutput")
    m_d = nc.dram_tensor("m_out", [BPC, NST, 128, KG], _s8, kind="ExternalOutput")

    with tile.TileContext(nc) as tc:
        with tc.tile_pool(name="const", bufs=1) as cpool, tc.tile_pool(
            name="io", bufs=2
        ) as iop, tc.tile_pool(name="st", bufs=3) as stp, tc.tile_pool(
            name="out", bufs=3
        ) as outp:
            # pin the act table that holds Ln+Exp+Sign so the auto-inserter
            # doesn't ping-pong between per-func tables each supertile
            _actload = mybir.InstLoadActFuncSet(
                name=nc.get_next_instruction_name(), ins=[], outs=[],
                act_func_set_id=ACT_TABLE_ID,
            )
            _actload.engine = mybir.EngineType.Activation
            nc.scalar.add_instruction(_actload)
            zspan = max(1, KG - min(gs) * K)
            zf = cpool.tile([128, zspan], _f32, tag="zf")
            zi = cpool.tile([128, zspan], _s8, tag="zi")
            nc.gpsimd.memset(zf[:], 0.0)
            nc.gpsimd.memset(zi[:], 0)

            io_tiles = {}

            def load_io(s):
                g = gs[s]
                gt_t = iop.tile([128, 5 * G], _f32, tag="gt")
                pf_t = iop.tile([128, NT * 5], _f32, tag="pf")
                nc.sync.dma_start(out=gt_t[:, : 5 * g], in_=gt[s, :, : 5 * g])
                nc.sync.dma_start(out=pf_t[:], in_=pf[s])
                io_tiles[s] = (gt_t, pf_t)

            def phase_a(s, st):
                """overlap customs + inter (DVE) + u1 (Pool)."""
                g = gs[s]
                fs = g * K
                gt_t, pf_t = io_tiles[s]
                gx1 = gt_t[:, 0:g]
                gx2 = gt_t[:, g : 2 * g]
                gy1 = gt_t[:, 2 * g : 3 * g]
                gy2 = gt_t[:, 3 * g : 4 * g]
                ag_b = gt_t[:, 4 * g : 5 * g].unsqueeze(2).broadcast_to([128, g, K])
                dxr = stp.tile([128, fs], _f32, tag="dxr")
                dyr = stp.tile([128, fs], _f32, tag="dyr")
                inter = stp.tile([128, fs], _f32, tag="inter")
                dxr3 = dxr[:].rearrange("p (g k) -> p g k", k=K)
                dyr3 = dyr[:].rearrange("p (g k) -> p g k", k=K)
                for k in range(K):
                    t = st * K + k
                    px1 = pf_t[:, t * 5 + 0 : t * 5 + 1]
                    px2 = pf_t[:, t * 5 + 1 : t * 5 + 2]
                    py1 = pf_t[:, t * 5 + 2 : t * 5 + 3]
                    py2 = pf_t[:, t * 5 + 3 : t * 5 + 4]
                    nc.vector._custom_dve(
                        IOU_DX, out=dxr3[:, :, k : k + 1].squeeze(2),
                        in0=gx2, in1=gx1, s0=px2, s1=px1,
                    )
                    nc.vector._custom_dve(
                        IOU_DX, out=dyr3[:, :, k : k + 1].squeeze(2),
                        in0=gy2, in1=gy1, s0=py2, s1=py1,
                    )
                nc.vector.tensor_mul(inter[:], dxr[:], dyr[:])
                return dxr, dyr, inter

            def phase_apg(s, st):
                """apg = ap + ag from broadcast APs — depends only on the
                slot's input tiles, so the scheduler can run it whenever the
                Pool engine would otherwise stall."""
                g = gs[s]
                fs = g * K
                gt_t, pf_t = io_tiles[s]
                ag_b = gt_t[:, 4 * g : 5 * g].unsqueeze(2).broadcast_to([128, g, K])
                apf_b = (
                    pf_t[:, st * K * 5 + 4 : (st + 1) * K * 5 : 5]
                    .unsqueeze(1)
                    .broadcast_to([128, g, K])
                )
                apg = stp.tile([128, fs], _f32, tag="apg")
                apg3 = apg[:].rearrange("p (g k) -> p g k", k=K)
                nc.gpsimd.tensor_tensor(apg3, apf_b, ag_b, _ALU.add)
                return apg

            def phase_b1(s, st, sti, tiles, apg):
                """union + msub + ln + exp."""
                g = gs[s]
                fs = g * K
                dxr, dyr, inter = tiles
                union = stp.tile([128, fs], _f32, tag="union")
                # union = (ap + ag) - inter — same rounding order as the
                # reference
                ueng = nc.vector if sti % 12 == 5 else nc.gpsimd
                ueng.tensor_tensor(union[:], apg[:], inter[:], _ALU.subtract)
                # mask path: int8 mask = sign(2*inter - union) via exact f32
                # compares, one DVE custom (Pool lacks stt; Act sign not needed)
                mm = outp.tile([128, fs], _s8, tag="mm")
                nc.vector._custom_dve(
                    IOU_MSIGN, out=mm[:], in0=inter[:], in1=union[:], s0=2.0
                )
                # value path: iou = inter * exp(-ln(union)); lnu lands in the
                # iou output tile, ru overwrites union (Ln was its last reader)
                iou = outp.tile([128, fs], _f32, tag="iou")
                lnu = iou
                nc.scalar.activation(lnu[:], union[:], _ACT.Ln)
                ru = union
                nc.scalar.activation(ru[:], lnu[:], _ACT.Exp, scale=-1.0)
                return mm, ru, iou

            def phase_b2(s, st, tiles, btiles):
                g = gs[s]
                fs = g * K
                _, _, inter = tiles
                mm, ru, iou = btiles
                nc.gpsimd.tensor_mul(iou[:], inter[:], ru[:])
                nc.sync.dma_start(out=iou_d[s, st, :, 0:fs], in_=iou[:])
                nc.sync.dma_start(out=m_d[s, st, :, 0:fs], in_=mm[:])
                if fs < KG:
                    nc.sync.dma_start(
                        out=iou_d[s, st, :, fs:KG], in_=zf[:, : KG - fs]
                    )
                    nc.sync.dma_start(
                        out=m_d[s, st, :, fs:KG], in_=zi[:, : KG - fs]
                    )

            # software pipeline with one-supertile skew; apg(i) is emitted an
            # iteration early as dependency-free Pool filler.
            slot_order = list(range(BPC))
            order_st = [(s, st) for s in slot_order for st in range(NST)]
            load_io(slot_order[0])
            apgs = {0: phase_apg(*order_st[0])}
            pending = None
            for i, (s, st) in enumerate(order_st):
                if st == NST - 1 and i + 1 < len(order_st):
                    load_io(order_st[i + 1][0])
                tiles = phase_a(s, st)
                if i + 1 < len(order_st):
                    apgs[i + 1] = phase_apg(*order_st[i + 1])
                if pending is not None:
                    pi, ps, pst, ptiles = pending
                    pbtiles = phase_b1(ps, pst, ps * NST + pst, ptiles, apgs.pop(pi))
                    phase_b2(ps, pst, ptiles, pbtiles)
                pending = (i, s, st, tiles)
            pi, ps, pst, ptiles = pending
            pbtiles = phase_b1(ps, pst, ps * NST + pst, ptiles, apgs.pop(pi))
            phase_b2(ps, pst, ptiles, pbtiles)
    nc.compile()
    return nc


def _get_nc(gs):
    key = tuple(gs)
    if key not in _NC_CACHE:
        _NC_CACHE[key] = _build_nc(key)
    return _NC_CACHE[key]


def kernel(
    threshhold=None,
    batch_boxes=None,
    batch_classes=None,
    batch_gt=None,
    batch_num_objects=None,
    **_kw,
):
    boxes = np.asarray(batch_boxes, np.float32)
    gtb = np.asarray(batch_gt, np.float32)
    no = np.asarray(batch_num_objects).astype(np.int64)

    half = np.float32(0.5)
    cx, cy, w, h = boxes[..., 0], boxes[..., 1], boxes[..., 2], boxes[..., 3]
    px1 = cx - w * half
    py1 = cy - h * half
    px2 = cx + w * half
    py2 = cy + h * half
    area_p = (px2 - px1) * (py2 - py1)

    def pad(a, fill):
        out = np.full((B, NPAD), fill, np.float32)
        out[:, :N] = a
        return out

    pf = np.stack(
        [pad(px1, -1e4), pad(px2, -1e4), pad(py1, -1e4), pad(py2, -1e4), pad(area_p, 1.0)],
        axis=-1,
    )  # [B, NPAD, 5]
    pf = np.ascontiguousarray(
        pf.reshape(B, NT, 128, 5).transpose(0, 2, 1, 3).reshape(B, 128, NT * 5)
    )

    gcx, gcy, gw, gh = gtb[..., 0], gtb[..., 1], gtb[..., 2], gtb[..., 3]
    gx1 = gcx - gw * half
    gy1 = gcy - gh * half
    gx2 = gcx + gw * half
    gy2 = gcy + gh * half
    area_g = (gx2 - gx1) * (gy2 - gy1)
    validm = np.arange(G)[None, :] < no[:, None]  # [B, G]
    NEG = np.float32(-1e6)
    gx1 = np.where(validm, gx1, NEG).astype(np.float32)
    gx2 = np.where(validm, gx2, NEG).astype(np.float32)
    gy1 = np.where(validm, gy1, NEG).astype(np.float32)
    gy2 = np.where(validm, gy2, NEG).astype(np.float32)
    area_g = np.where(validm, area_g, np.float32(0.0)).astype(np.float32)

    # sort batches by num_objects desc; slot s takes ranks [s*8, s*8+8)
    order = np.argsort(-no, kind="stable")
    gs = []
    for s in range(BPC):
        mx = int(no[order[s * NCORES : (s + 1) * NCORES]].max())
        mx = min(G, max(8, -(-mx // 4) * 4))
        gs.append(mx)
    gs = tuple(gs)

    # gt pack per batch: [gx1 | gx2 | gy1 | gy2 | ag] each g_s wide
    gtpack = np.zeros((B, 5 * G), np.float32)
    slot_of = np.empty(B, np.int64)
    for rank, b in enumerate(order):
        slot_of[b] = rank // NCORES
    for b in range(B):
        g = gs[slot_of[b]]
        gtpack[b, 0 * g : 1 * g] = gx1[b, :g]
        gtpack[b, 1 * g : 2 * g] = gx2[b, :g]
        gtpack[b, 2 * g : 3 * g] = gy1[b, :g]
        gtpack[b, 3 * g : 4 * g] = gy2[b, :g]
        gtpack[b, 4 * g : 5 * g] = area_g[b, :g]
    gtrep = np.broadcast_to(gtpack[:, None, :], (B, 128, 5 * G))

    nc = _get_nc(gs)
    in_maps = []
    for c in range(NCORES):
        bidx = [int(order[s * NCORES + c]) for s in range(BPC)]
        in_maps.append(
            {
                "pf": np.ascontiguousarray(pf[bidx]),
                "gt": np.ascontiguousarray(gtrep[bidx]),
            }
        )
    trace = os.environ.get("IOU_TRACE", "0") == "1"
    res = run_bass_kernel_spmd(nc, in_maps, list(range(NCORES)), trace=trace)
    _NC_CACHE["last_result"] = res
    results = res.results

    def unscramble(a):
        # [BPC, NST, 128, G*K] g-major -> [BPC, NPAD, G]; n = (st*K+k)*128 + p
        a = a.reshape(BPC, NST, 128, G, K).transpose(0, 1, 4, 2, 3)
        return a.reshape(BPC, NPAD, G)

    iou_full = np.empty((B, N, G), np.float32)
    m_full = np.empty((B, N, G), np.int8)
    for c in range(NCORES):
        r = results[c]
        iu = unscramble(r["iou_out"])
        mu = unscramble(r["m_out"])
        for s in range(BPC):
            b = int(order[s * NCORES + c])
            iou_full[b] = iu[s, :N]
            m_full[b] = mu[s, :N]
    vb = validm[:, None, :]
    pos = (m_full > 0) & vb
    neg = (m_full < 0) & vb
    return iou_full, pos, neg
